# revision 13
# baseline (speedup 1.0000x reference)
"""MixLinear GEMM kernel for Trainium2 (8 NeuronCores, column-parallel).

Computes, for full inputs:
    inputs = x.reshape(-1, 4096)
    act_outliers = inputs[:, ind]
    inputs_z = inputs with ind-columns zeroed
    x_scale = clamp(rowmax(|inputs_z|)/127, 1e-8)
    q_x = round(inputs_z / x_scale)                  (|q_x| <= 127 by construction)
    y = (q_x @ q_weight.T) * x_scale * scale_col + act_outliers @ weight_cache.T + bias

Device-side formulation: the host pre-packs a combined bf16 weight
    Wc[k, o] = q_weight[o, k] * scale_col[o]          for k not in ind
    Wc[k, o] = sum_{j: ind[j]==k} weight_cache[o, j]  for k in ind
so that with q~[m, k] = round(x[m, k] / xs[m]) (UNMASKED - outlier columns
carry the rounded outlier activation) the output is simply
    y[m, o] = (sum_k q~[m, k] * Wc[k, o]) * xs[m] + bias[o].

v2 (this file): the host supplies x in BOTH layouts - [M, K] for the
row-absmax, and transposed [K, M] so q~^T is produced directly in the
[k-partition, m-free] layout the matmul needs. This removes the device
DMA transposes entirely (v1's main stall: 2048 256-byte descriptors per
transpose saturated all 16 DMA queues right when the PE needed q).

q~^T is produced per k-group: DVE multiplies xT by a broadcast recip row
(fp32), then ACT adds 1536 storing fp16 (RNE snaps to an exact integer:
fp16 spacing is 1.0 in [1024, 2048)), then subtracts 1536 storing bf16.
The recip row broadcast [128, 512] is built via a tiny DRAM round-trip
(stage [128, MT] then re-read with a partition-crossing access pattern).

Engine placement per rep: Pool does the masked absmax (mask-mult +
abs-max reduce); DVE does the scale finals, the xT*recip multiplies and
the output epilogues; ACT does the two magic-round passes; SP issues
x/xT/y/recip DMAs; Pool also issues the weight-stream DMAs; PE runs the
matmuls with all 8 PSUM banks in flight.

Emission is software-pipelined and INTERLEAVED at chunk granularity:
  phase2(r, c0) | phase1_absmax(r+1) | phase2(r, c1) | phase1_quant(r+1)
  | phase2(r, c2)
so no engine queue has a rep's worth of head-of-line blocking (v1 put
all of phase1(r+1) ahead of phase2(r)'s epilogues on DVE, which starved
the PSUM drain and stalled the PE mid-rep).
"""

import sys

import numpy as np

sys.path.insert(0, "/opt/trn_rl_repo")

import concourse.bass as bass  # noqa: E402
import concourse.mybir as mybir  # noqa: E402
import concourse.tile as tile  # noqa: E402
from concourse import bacc  # noqa: E402

N_CORES = 8
M = 512  # 8*64 rows
K = 4096  # in_features
OUT = 11008  # out_features
OSH = OUT // N_CORES  # 1376 per-core shard
FP = 256  # outlier columns
KT = K // 128  # 32 k-tiles
MT = M // 128  # 4 m-tiles
MAGIC = 1536.0  # fp16 spacing is 1.0 in [1024, 2048): forces round-to-int
OC = 459  # padded o-chunk width (fits one PSUM bank: 459*4B <= 2KB)
OCS = [459, 459, 458]  # actual chunk widths (sum = OSH)
OFF = [0, 459, 918]
NCH = 3  # chunks
XH = 2048  # x streamed in half-rows for the absmax
KG = 4  # k-tiles per quantize group
NKG = KT // KG  # 8 groups

f32 = mybir.dt.float32
f16 = mybir.dt.float16
bf16 = mybir.dt.bfloat16
Alu = mybir.AluOpType
Act = mybir.ActivationFunctionType


def build_program(nrep=1):
    """Build the kernel program. nrep>1 emits the whole body nrep times
    (same inputs, same outputs) - used only to measure steady-state HW time
    as (t(nrep) - t(1)) / (nrep - 1)."""
    nc = bacc.Bacc(
        "TRN2", target_bir_lowering=False, debug=False, num_devices=N_CORES
    )

    x_d = nc.dram_tensor("x_in", [M, K], f16, kind="ExternalInput").ap()
    xt_d = nc.dram_tensor("xt_in", [K, M], f16, kind="ExternalInput").ap()
    # host-packed combined weight: [chunk, partition(k%128), kk, o-in-chunk]
    w_d = nc.dram_tensor("w_in", [NCH, 128, KT * OC], bf16, kind="ExternalInput").ap()
    mask_d = nc.dram_tensor("mask_in", [1, K], f16, kind="ExternalInput").ap()
    bias_d = nc.dram_tensor("bias_in", [1, OSH], bf16, kind="ExternalInput").ap()
    y_d = nc.dram_tensor("y_out", [M, OSH], f16, kind="ExternalOutput").ap()
    # recip staging for the row-broadcast (4 slots, rep%4-indexed)
    rs_d = nc.dram_tensor("rs", [128, 4 * MT], f32, kind="Internal").ap()

    with tile.TileContext(nc) as tc:
        with (
            tc.tile_pool(name="persist", bufs=1) as persist,
            tc.tile_pool(name="xpool", bufs=2) as xpool,
            tc.tile_pool(name="xzpool", bufs=2) as xzpool,
            tc.tile_pool(name="xtpool", bufs=2) as xtpool,
            tc.tile_pool(name="tpool", bufs=1) as tpool,
            tc.tile_pool(name="qnpool", bufs=1) as qnpool,
            tc.tile_pool(name="rbpool", bufs=2) as rbpool,
            tc.tile_pool(name="wtpool", bufs=3) as wtpool,
            tc.tile_pool(name="ypool", bufs=2) as ypool,
            tc.tile_pool(name="psmain", bufs=8, space="PSUM") as psmain,
        ):
            # ---------- persistent tiles ----------
            # q~^T [k-part, kk, m]: one tile per rep parity
            q_sets = [
                persist.tile([128, KT, M], bf16, tag=f"qT{par}", name=f"qT{par}")
                for par in range(2)
            ]
            mask_bc = persist.tile([128, K], f16)  # ind-mask broadcast
            bias_bc = persist.tile([128, OSH], bf16)  # bias broadcast
            am_parts = persist.tile([128, MT * 2], f32)
            am_all = persist.tile([128, MT], f32)
            xs_all = persist.tile([128, 4 * MT], f32)  # rep%4-indexed
            recip_all = persist.tile([128, 4 * MT], f32)

            # ---------- setup ----------
            nc.gpsimd.dma_start(
                out=mask_bc,
                in_=bass.AP(mask_d.tensor, mask_d.offset, [[0, 128], [1, K]]),
            )
            nc.gpsimd.dma_start(
                out=bias_bc,
                in_=bass.AP(bias_d.tensor, bias_d.offset, [[0, 128], [1, OSH]]),
            )

            recip_bcs = {}

            def phase1_absmax(rep):
                """x loads, masked absmax (Pool), scale finals (DVE), and
                the recip row-broadcast DRAM round-trip (SP)."""
                pq = rep % 4
                for mt in range(MT):
                    ms = slice(mt * 128, (mt + 1) * 128)
                    xhs = []
                    for h in range(2):
                        x_h = xpool.tile(
                            [128, XH], f16, tag="x", name=f"x_{rep}_{mt}_{h}"
                        )
                        nc.sync.dma_start(
                            out=x_h, in_=x_d[ms, h * XH : (h + 1) * XH]
                        )
                        xhs.append(x_h)
                    for h in range(2):
                        # mask-mult on Pool, abs-max row reduce on DVE
                        xz = xzpool.tile(
                            [128, XH], f16, tag="xz", name=f"xz_{rep}_{mt}_{h}"
                        )
                        nc.gpsimd.tensor_tensor(
                            out=xz,
                            in0=xhs[h],
                            in1=mask_bc[:, h * XH : (h + 1) * XH],
                            op=Alu.mult,
                        )
                        nc.vector.tensor_reduce(
                            out=am_parts[:, mt * 2 + h : mt * 2 + h + 1],
                            in_=xz,
                            axis=mybir.AxisListType.X,
                            op=Alu.max,
                            apply_absolute_value=True,
                        )
                    nc.vector.tensor_reduce(
                        out=am_all[:, mt : mt + 1],
                        in_=am_parts[:, mt * 2 : mt * 2 + 2],
                        axis=mybir.AxisListType.X,
                        op=Alu.max,
                        apply_absolute_value=False,
                    )
                    pc = pq * MT + mt
                    # xs = max(absmax/127, 1e-8); recip = 1/xs
                    nc.vector.tensor_scalar(
                        xs_all[:, pc : pc + 1],
                        am_all[:, mt : mt + 1],
                        1.0 / 127.0,
                        1e-8,
                        Alu.mult,
                        Alu.max,
                    )
                    nc.vector.reciprocal(
                        out=recip_all[:, pc : pc + 1], in_=xs_all[:, pc : pc + 1]
                    )
                # stage recip [128, MT] to DRAM slot pq, re-read as a
                # broadcast row [128, mt, m%128] = recip[m]
                nc.sync.dma_start(
                    out=rs_d[:, pq * MT : (pq + 1) * MT],
                    in_=recip_all[:, pq * MT : (pq + 1) * MT],
                )
                rb = rbpool.tile([128, M], f32, tag="rb", name=f"rb_{rep}")
                for mt in range(MT):
                    nc.sync.dma_start(
                        out=rb[:, mt * 128 : (mt + 1) * 128],
                        in_=bass.AP(
                            rs_d.tensor,
                            rs_d.offset + pq * MT + mt,
                            [[0, 128], [4 * MT, 128]],
                        ),
                    )
                recip_bcs[rep] = rb

            def phase1_quant(rep):
                """xT loads (SP), xT*recip (DVE), two magic-round passes
                (ACT) into this rep's q~^T set."""
                par = rep % 2
                q_t = q_sets[par]
                rb = recip_bcs.pop(rep)
                for g in range(NKG):
                    xt = xtpool.tile(
                        [128, KG, M], f16, tag="xt", name=f"xt_{rep}_{g}"
                    )
                    for j in range(KG):
                        kk = g * KG + j
                        nc.sync.dma_start(
                            out=xt[:, j, :],
                            in_=xt_d[kk * 128 : (kk + 1) * 128, :],
                        )
                    t32 = tpool.tile(
                        [128, KG, M], f32, tag="t32", name=f"t32_{rep}_{g}"
                    )
                    nc.vector.tensor_tensor(
                        out=t32,
                        in0=xt,
                        in1=rb.unsqueeze(1).broadcast_to((128, KG, M)),
                        op=Alu.mult,
                    )
                    qn = qnpool.tile(
                        [128, KG, M], f16, tag="qn", name=f"qn_{rep}_{g}"
                    )
                    nc.scalar.activation(
                        out=qn, in_=t32, func=Act.Copy, bias=MAGIC, scale=1.0
                    )
                    nc.scalar.activation(
                        out=q_t[:, g * KG : (g + 1) * KG, :],
                        in_=qn,
                        func=Act.Copy,
                        bias=-MAGIC,
                        scale=1.0,
                    )

            def load_w(rep, c):
                wt = wtpool.tile(
                    [128, KT, OC], bf16, tag="wt", name=f"wt_{rep}_{c}"
                )
                nc.gpsimd.dma_start(out=wt, in_=w_d[c])
                return wt

            wt_cur = {}
            wt_next = {}

            def phase2_chunk(rep, c, prefetch_next):
                par = rep % 2
                pq = rep % 4
                q_t = q_sets[par]
                nonlocal wt_cur, wt_next
                wt = wt_cur[c]
                o0 = OFF[c]
                cw = OCS[c]
                for mt in range(MT):
                    ms = slice(mt * 128, (mt + 1) * 128)
                    pc = pq * MT + mt
                    ps = psmain.tile(
                        [128, OC], f32, tag="ps", name=f"ps_{rep}_{c}_{mt}"
                    )
                    for kk in range(KT):
                        nc.tensor.matmul(
                            ps,
                            lhsT=q_t[:, kk, mt * 128 : (mt + 1) * 128],
                            rhs=wt[:, kk, :],
                            start=(kk == 0),
                            stop=(kk == KT - 1),
                        )
                    ysb = ypool.tile(
                        [128, OC], f16, tag="ysb", name=f"ysb_{rep}_{c}_{mt}"
                    )
                    # y = ps * xs + bias
                    nc.vector.scalar_tensor_tensor(
                        out=ysb[:, :cw],
                        in0=ps[:, :cw],
                        scalar=xs_all[:, pc : pc + 1],
                        in1=bias_bc[:, o0 : o0 + cw],
                        op0=Alu.mult,
                        op1=Alu.add,
                    )
                    nc.sync.dma_start(out=y_d[ms, o0 : o0 + cw], in_=ysb[:, :cw])
                # prefetch next rep's chunk-c weights now that this rep's
                # reads of the same wt buffer slot are emitted
                if prefetch_next:
                    wt_next[c] = load_w(rep + 1, c)
                if c == NCH - 1 and prefetch_next:
                    wt_cur = wt_next
                    wt_next = {}

            # software-pipelined, chunk-interleaved emission
            wt_cur = {cc: load_w(0, cc) for cc in range(NCH)}
            phase1_absmax(0)
            phase1_quant(0)
            for rep in range(nrep):
                more = rep + 1 < nrep
                if more:
                    # absmax first: depends only on x(r+1) loads, so DVE/Pool
                    # start immediately at rep start and recip is ready early
                    phase1_absmax(rep + 1)
                phase2_chunk(rep, 0, prefetch_next=more)
                if more:
                    phase1_quant(rep + 1)
                phase2_chunk(rep, 1, prefetch_next=more)
                phase2_chunk(rep, 2, prefetch_next=more)

    nc.compile()
    return nc


_NC_CACHE = None


def get_program():
    global _NC_CACHE
    if _NC_CACHE is None:
        _NC_CACHE = build_program()
    return _NC_CACHE


def make_in_maps(x, q_weight, scale_col, weight_cache, ind, bias):
    x2 = np.ascontiguousarray(
        np.asarray(x, dtype=np.float32).reshape(M, K).astype(np.float16)
    )
    xt = np.ascontiguousarray(x2.T)
    q_weight = np.asarray(q_weight, dtype=np.int32)
    scale_col = np.asarray(scale_col, dtype=np.float32).reshape(OUT)
    weight_cache = np.asarray(weight_cache, dtype=np.float32)
    ind_np = np.asarray(ind, dtype=np.int32).reshape(FP)
    bias_np = np.asarray(bias, dtype=np.float32).reshape(OUT)

    import ml_dtypes

    mask = np.ones(K, dtype=np.float32)
    mask[ind_np] = 0.0
    mask_bf = mask.astype(np.float16).reshape(1, K)

    # combined weight: WcT[k, o] = q_weight[o, k]*scale_col[o] off-outlier,
    # scatter-add of weight_cache on outlier rows (duplicates in ind add,
    # matching x[:, ind] gather + separate GEMM in the reference)
    wf = q_weight.astype(np.float32) * scale_col.reshape(OUT, 1)  # [OUT, K]
    wcT = np.ascontiguousarray(wf.T)  # [K, OUT]
    cr = np.zeros((K, OUT), dtype=np.float32)
    np.add.at(cr, ind_np, weight_cache.T.astype(np.float32))
    outlier_rows = np.zeros(K, dtype=bool)
    outlier_rows[ind_np] = True
    wcT[outlier_rows] = cr[outlier_rows]
    wc16 = wcT.astype(ml_dtypes.bfloat16)  # [K, OUT]

    in_maps = []
    for core in range(N_CORES):
        sl = slice(core * OSH, (core + 1) * OSH)
        shard = wc16[:, sl]  # [K, OSH]
        # pack: [chunk, partition(k%128), kk, o-in-chunk], zero-padded to OC
        wpack = np.zeros((NCH, 128, KT, OC), dtype=wc16.dtype)
        r = shard.reshape(KT, 128, OSH)
        for c in range(NCH):
            wpack[c, :, :, : OCS[c]] = r[:, :, OFF[c] : OFF[c] + OCS[c]].transpose(
                1, 0, 2
            )
        wpack = np.ascontiguousarray(wpack).reshape(NCH, 128, KT * OC)
        in_maps.append(
            {
                "x_in": x2,
                "xt_in": xt,
                "w_in": wpack,
                "mask_in": mask_bf,
                "bias_in": np.ascontiguousarray(
                    bias_np[sl].astype(ml_dtypes.bfloat16).reshape(1, OSH)
                ),
            }
        )
    return in_maps


def kernel(x, q_weight, scale_col, weight_cache, ind, bias):
    from concourse.bass_utils import run_bass_kernel_spmd

    nc = get_program()
    in_maps = make_in_maps(x, q_weight, scale_col, weight_cache, ind, bias)
    res = run_bass_kernel_spmd(nc, in_maps, core_ids=list(range(N_CORES)))
    shards = [res.results[c]["y_out"] for c in range(N_CORES)]
    y = np.concatenate(shards, axis=1)
    return y.reshape(8, 64, OUT).astype(np.float32)


# revision 22
# speedup vs baseline: 1.5467x; 1.5467x over previous
"""MixLinear GEMM kernel for Trainium2 (8 NeuronCores, column-parallel).

Computes, for full inputs:
    inputs = x.reshape(-1, 4096)
    act_outliers = inputs[:, ind]
    inputs_z = inputs with ind-columns zeroed
    x_scale = clamp(rowmax(|inputs_z|)/127, 1e-8)
    q_x = round(inputs_z / x_scale)                  (|q_x| <= 127 by construction)
    y = (q_x @ q_weight.T) * x_scale * scale_col + act_outliers @ weight_cache.T + bias

Device-side formulation: the host pre-packs a combined bf16 weight
    Wc[k, o] = q_weight[o, k] * scale_col[o]          for k not in ind
    Wc[k, o] = sum_{j: ind[j]==k} weight_cache[o, j]  for k in ind
so that with q~[m, k] = round(x[m, k] / xs[m]) (UNMASKED - outlier columns
carry the rounded outlier activation) the output is simply
    y[m, o] = (sum_k q~[m, k] * Wc[k, o]) * xs[m] + bias[o].

v2 (this file): the host supplies x in BOTH layouts - [M, K] for the
row-absmax, and transposed [K, M] so q~^T is produced directly in the
[k-partition, m-free] layout the matmul needs. This removes the device
DMA transposes entirely (v1's main stall: 2048 256-byte descriptors per
transpose saturated all 16 DMA queues right when the PE needed q).

q~^T is produced per k-group: DVE multiplies xT by a broadcast recip row
(fp32), then ACT adds 1536 storing fp16 (RNE snaps to an exact integer:
fp16 spacing is 1.0 in [1024, 2048)), then subtracts 1536 storing bf16.
The recip row broadcast [128, 512] is built via a tiny DRAM round-trip
(stage [128, MT] then re-read with a partition-crossing access pattern).

Engine placement per rep: Pool does the masked absmax (mask-mult +
abs-max reduce); DVE does the scale finals, the xT*recip multiplies and
the output epilogues; ACT does the two magic-round passes; SP issues
x/xT/y/recip DMAs; Pool also issues the weight-stream DMAs; PE runs the
matmuls with all 8 PSUM banks in flight.

Emission is software-pipelined and INTERLEAVED at chunk granularity:
  phase2(r, c0) | phase1_absmax(r+1) | phase2(r, c1) | phase1_quant(r+1)
  | phase2(r, c2)
so no engine queue has a rep's worth of head-of-line blocking (v1 put
all of phase1(r+1) ahead of phase2(r)'s epilogues on DVE, which starved
the PSUM drain and stalled the PE mid-rep).
"""

import sys

import numpy as np

sys.path.insert(0, "/opt/trn_rl_repo")

import concourse.bass as bass  # noqa: E402
import concourse.mybir as mybir  # noqa: E402
import concourse.tile as tile  # noqa: E402
from concourse import bacc  # noqa: E402

N_CORES = 8
M = 512  # 8*64 rows
K = 4096  # in_features
OUT = 11008  # out_features
OSH = OUT // N_CORES  # 1376 per-core shard
FP = 256  # outlier columns
KT = K // 128  # 32 k-tiles
MT = M // 128  # 4 m-tiles
MAGIC = 1536.0  # fp16 spacing is 1.0 in [1024, 2048): forces round-to-int
OC = 459  # padded o-chunk width (fits one PSUM bank: 459*4B <= 2KB)
OCS = [459, 459, 458]  # actual chunk widths (sum = OSH)
OFF = [0, 459, 918]
NCH = 3  # chunks
XH = 2048  # x streamed in half-rows for the absmax
KG = 4  # k-tiles per quantize group
NKG = KT // KG  # 8 groups

f32 = mybir.dt.float32
f16 = mybir.dt.float16
bf16 = mybir.dt.bfloat16
Alu = mybir.AluOpType
Act = mybir.ActivationFunctionType


def build_program(nrep=1):
    """Build the kernel program. nrep>1 emits the whole body nrep times
    (same inputs, same outputs) - used only to measure steady-state HW time
    as (t(nrep) - t(1)) / (nrep - 1)."""
    nc = bacc.Bacc(
        "TRN2", target_bir_lowering=False, debug=False, num_devices=N_CORES
    )

    x_d = nc.dram_tensor("x_in", [M, K], f16, kind="ExternalInput").ap()
    xt_d = nc.dram_tensor("xt_in", [K, M], f16, kind="ExternalInput").ap()
    # host-packed combined weight: [chunk, partition(k%128), kk, o-in-chunk]
    w_d = nc.dram_tensor("w_in", [NCH, 128, KT * OC], bf16, kind="ExternalInput").ap()
    mask_d = nc.dram_tensor("mask_in", [1, K], f16, kind="ExternalInput").ap()
    bias_d = nc.dram_tensor("bias_in", [1, OSH], bf16, kind="ExternalInput").ap()
    y_d = nc.dram_tensor("y_out", [M, OSH], f16, kind="ExternalOutput").ap()
    # recip staging for the row-broadcast (4 slots, rep%4-indexed),
    # stored transposed: rs[slot, m] = recip[m]
    rs_d = nc.dram_tensor("rs", [4, M], f32, kind="Internal").ap()

    with tile.TileContext(nc) as tc:
        with (
            tc.tile_pool(name="persist", bufs=1) as persist,
            tc.tile_pool(name="xpool", bufs=2) as xpool,
            tc.tile_pool(name="xzpool", bufs=2) as xzpool,
            tc.tile_pool(name="xtpool", bufs=2) as xtpool,
            tc.tile_pool(name="tpool", bufs=1) as tpool,
            tc.tile_pool(name="qnpool", bufs=1) as qnpool,
            tc.tile_pool(name="rbpool", bufs=1) as rbpool,
            tc.tile_pool(name="wtpool", bufs=3) as wtpool,
            tc.tile_pool(name="ypool", bufs=2) as ypool,
            tc.tile_pool(name="psmain", bufs=8, space="PSUM") as psmain,
        ):
            # ---------- persistent tiles ----------
            # q~^T [k-part, kk, m]: one tile per rep parity
            q_sets = [
                persist.tile([128, KT, M], bf16, tag=f"qT{par}", name=f"qT{par}")
                for par in range(2)
            ]
            mask_bc = persist.tile([128, K], f16)  # ind-mask broadcast
            bias_bc = persist.tile([128, OSH], bf16)  # bias broadcast
            am_parts = persist.tile([128, MT * 2], f32)
            am_all = persist.tile([128, MT], f32)
            xs_all = persist.tile([128, 4 * MT], f32)  # rep%4-indexed
            recip_all = persist.tile([128, 4 * MT], f32)

            # ---------- setup ----------
            nc.gpsimd.dma_start(
                out=mask_bc,
                in_=bass.AP(mask_d.tensor, mask_d.offset, [[0, 128], [1, K]]),
            )
            nc.gpsimd.dma_start(
                out=bias_bc,
                in_=bass.AP(bias_d.tensor, bias_d.offset, [[0, 128], [1, OSH]]),
            )

            recip_bcs = {}

            def phase1_absmax(rep):
                """x loads, masked absmax (Pool), scale finals (DVE), and
                the recip row-broadcast DRAM round-trip (SP)."""
                pq = rep % 4
                for mt in range(MT):
                    ms = slice(mt * 128, (mt + 1) * 128)
                    xhs = []
                    for h in range(2):
                        x_h = xpool.tile(
                            [128, XH], f16, tag="x", name=f"x_{rep}_{mt}_{h}"
                        )
                        nc.gpsimd.dma_start(
                            out=x_h, in_=x_d[ms, h * XH : (h + 1) * XH]
                        )
                        xhs.append(x_h)
                    for h in range(2):
                        # mask-mult on Pool, abs-max row reduce on DVE
                        xz = xzpool.tile(
                            [128, XH], f16, tag="xz", name=f"xz_{rep}_{mt}_{h}"
                        )
                        nc.gpsimd.tensor_tensor(
                            out=xz,
                            in0=xhs[h],
                            in1=mask_bc[:, h * XH : (h + 1) * XH],
                            op=Alu.mult,
                        )
                        nc.vector.tensor_reduce(
                            out=am_parts[:, mt * 2 + h : mt * 2 + h + 1],
                            in_=xz,
                            axis=mybir.AxisListType.X,
                            op=Alu.max,
                            apply_absolute_value=True,
                        )
                    nc.vector.tensor_reduce(
                        out=am_all[:, mt : mt + 1],
                        in_=am_parts[:, mt * 2 : mt * 2 + 2],
                        axis=mybir.AxisListType.X,
                        op=Alu.max,
                        apply_absolute_value=False,
                    )
                    pc = pq * MT + mt
                    # xs = max(absmax/127, 1e-8); recip = 1/xs
                    nc.vector.tensor_scalar(
                        xs_all[:, pc : pc + 1],
                        am_all[:, mt : mt + 1],
                        1.0 / 127.0,
                        1e-8,
                        Alu.mult,
                        Alu.max,
                    )
                    nc.vector.reciprocal(
                        out=recip_all[:, pc : pc + 1], in_=xs_all[:, pc : pc + 1]
                    )
                # stage recip to DRAM slot pq in m-order (scatter on write:
                # rs[pq, mt*128 + p] = recip_all[p, pq*MT + mt]), then re-read
                # as a clean partition-broadcast row
                nc.sync.dma_start(
                    out=bass.AP(
                        rs_d.tensor,
                        rs_d.offset + pq * M,
                        [[1, 128], [128, MT]],
                    ),
                    in_=recip_all[:, pq * MT : (pq + 1) * MT],
                )
                rb = rbpool.tile([128, M], f32, tag="rb", name=f"rb_{rep}")
                nc.sync.dma_start(
                    out=rb,
                    in_=bass.AP(
                        rs_d.tensor, rs_d.offset + pq * M, [[0, 128], [1, M]]
                    ),
                )
                recip_bcs[rep] = rb

            def phase1_quant(rep):
                """xT loads (SP), xT*recip (DVE), two magic-round passes
                (ACT) into this rep's q~^T set."""
                par = rep % 2
                q_t = q_sets[par]
                rb = recip_bcs.pop(rep)
                # broadcast view of rb: [128, KG, M] with the KG dim stride-0
                rb_bc = bass.AP(
                    rb.tensor, rb.offset, [[M, 128], [0, KG], [1, M]]
                )
                for g in range(NKG):
                    xt = xtpool.tile(
                        [128, KG, M], f16, tag="xt", name=f"xt_{rep}_{g}"
                    )
                    # one 3D-AP DMA per group: tile[p, j, m] = xT[(g*KG+j)*128 + p, m]
                    nc.sync.dma_start(
                        out=xt,
                        in_=bass.AP(
                            xt_d.tensor,
                            xt_d.offset + g * KG * 128 * M,
                            [[M, 128], [128 * M, KG], [1, M]],
                        ),
                    )
                    t32 = tpool.tile(
                        [128, KG, M], f32, tag="t32", name=f"t32_{rep}_{g}"
                    )
                    nc.vector.tensor_tensor(
                        out=t32,
                        in0=xt,
                        in1=rb_bc,
                        op=Alu.mult,
                    )
                    qn = qnpool.tile(
                        [128, KG, M], f16, tag="qn", name=f"qn_{rep}_{g}"
                    )
                    nc.scalar.activation(
                        out=qn, in_=t32, func=Act.Copy, bias=MAGIC, scale=1.0
                    )
                    nc.scalar.activation(
                        out=q_t[:, g * KG : (g + 1) * KG, :],
                        in_=qn,
                        func=Act.Copy,
                        bias=-MAGIC,
                        scale=1.0,
                    )

            def load_w(rep, c):
                wt = wtpool.tile(
                    [128, KT, OC], bf16, tag="wt", name=f"wt_{rep}_{c}"
                )
                nc.gpsimd.dma_start(out=wt, in_=w_d[c])
                return wt

            wt_cur = {}
            wt_next = {}

            def phase2_chunk(rep, c, prefetch_next):
                par = rep % 2
                pq = rep % 4
                q_t = q_sets[par]
                nonlocal wt_cur, wt_next
                wt = wt_cur[c]
                o0 = OFF[c]
                cw = OCS[c]
                ysb = ypool.tile(
                    [128, MT, OC], f16, tag="ysb", name=f"ysb_{rep}_{c}"
                )
                for mt in range(MT):
                    pc = pq * MT + mt
                    ps = psmain.tile(
                        [128, OC], f32, tag="ps", name=f"ps_{rep}_{c}_{mt}"
                    )
                    for kk in range(KT):
                        nc.tensor.matmul(
                            ps,
                            lhsT=q_t[:, kk, mt * 128 : (mt + 1) * 128],
                            rhs=wt[:, kk, :],
                            start=(kk == 0),
                            stop=(kk == KT - 1),
                        )
                    # y = ps * xs + bias
                    nc.vector.scalar_tensor_tensor(
                        out=ysb[:, mt, :cw],
                        in0=ps[:, :cw],
                        scalar=xs_all[:, pc : pc + 1],
                        in1=bias_bc[:, o0 : o0 + cw],
                        op0=Alu.mult,
                        op1=Alu.add,
                    )
                # one 3D store for all 4 m-tiles of this chunk:
                # y[mt*128 + p, o0 + o] = ysb[p, mt, o]
                nc.scalar.dma_start(
                    out=bass.AP(
                        y_d.tensor,
                        y_d.offset + o0,
                        [[OSH, 128], [128 * OSH, MT], [1, cw]],
                    ),
                    in_=ysb[:, :, :cw],
                )
                # prefetch next rep's chunk-c weights now that this rep's
                # reads of the same wt buffer slot are emitted
                if prefetch_next:
                    wt_next[c] = load_w(rep + 1, c)
                if c == NCH - 1 and prefetch_next:
                    wt_cur = wt_next
                    wt_next = {}

            # software-pipelined, chunk-interleaved emission
            wt_cur = {cc: load_w(0, cc) for cc in range(NCH)}
            phase1_absmax(0)
            phase1_quant(0)
            for rep in range(nrep):
                more = rep + 1 < nrep
                if more:
                    # absmax first: depends only on x(r+1) loads, so DVE/Pool
                    # start immediately at rep start and recip is ready early
                    phase1_absmax(rep + 1)
                phase2_chunk(rep, 0, prefetch_next=more)
                if more:
                    phase1_quant(rep + 1)
                phase2_chunk(rep, 1, prefetch_next=more)
                phase2_chunk(rep, 2, prefetch_next=more)

    nc.compile()
    return nc


_NC_CACHE = None


def get_program():
    global _NC_CACHE
    if _NC_CACHE is None:
        _NC_CACHE = build_program()
    return _NC_CACHE


def make_in_maps(x, q_weight, scale_col, weight_cache, ind, bias):
    x2 = np.ascontiguousarray(
        np.asarray(x, dtype=np.float32).reshape(M, K).astype(np.float16)
    )
    xt = np.ascontiguousarray(x2.T)
    q_weight = np.asarray(q_weight, dtype=np.int32)
    scale_col = np.asarray(scale_col, dtype=np.float32).reshape(OUT)
    weight_cache = np.asarray(weight_cache, dtype=np.float32)
    ind_np = np.asarray(ind, dtype=np.int32).reshape(FP)
    bias_np = np.asarray(bias, dtype=np.float32).reshape(OUT)

    import ml_dtypes

    mask = np.ones(K, dtype=np.float32)
    mask[ind_np] = 0.0
    mask_bf = mask.astype(np.float16).reshape(1, K)

    # combined weight: WcT[k, o] = q_weight[o, k]*scale_col[o] off-outlier,
    # scatter-add of weight_cache on outlier rows (duplicates in ind add,
    # matching x[:, ind] gather + separate GEMM in the reference)
    wf = q_weight.astype(np.float32) * scale_col.reshape(OUT, 1)  # [OUT, K]
    wcT = np.ascontiguousarray(wf.T)  # [K, OUT]
    cr = np.zeros((K, OUT), dtype=np.float32)
    np.add.at(cr, ind_np, weight_cache.T.astype(np.float32))
    outlier_rows = np.zeros(K, dtype=bool)
    outlier_rows[ind_np] = True
    wcT[outlier_rows] = cr[outlier_rows]
    wc16 = wcT.astype(ml_dtypes.bfloat16)  # [K, OUT]

    in_maps = []
    for core in range(N_CORES):
        sl = slice(core * OSH, (core + 1) * OSH)
        shard = wc16[:, sl]  # [K, OSH]
        # pack: [chunk, partition(k%128), kk, o-in-chunk], zero-padded to OC
        wpack = np.zeros((NCH, 128, KT, OC), dtype=wc16.dtype)
        r = shard.reshape(KT, 128, OSH)
        for c in range(NCH):
            wpack[c, :, :, : OCS[c]] = r[:, :, OFF[c] : OFF[c] + OCS[c]].transpose(
                1, 0, 2
            )
        wpack = np.ascontiguousarray(wpack).reshape(NCH, 128, KT * OC)
        in_maps.append(
            {
                "x_in": x2,
                "xt_in": xt,
                "w_in": wpack,
                "mask_in": mask_bf,
                "bias_in": np.ascontiguousarray(
                    bias_np[sl].astype(ml_dtypes.bfloat16).reshape(1, OSH)
                ),
            }
        )
    return in_maps


def kernel(x, q_weight, scale_col, weight_cache, ind, bias):
    from concourse.bass_utils import run_bass_kernel_spmd

    nc = get_program()
    in_maps = make_in_maps(x, q_weight, scale_col, weight_cache, ind, bias)
    res = run_bass_kernel_spmd(nc, in_maps, core_ids=list(range(N_CORES)))
    shards = [res.results[c]["y_out"] for c in range(N_CORES)]
    y = np.concatenate(shards, axis=1)
    return y.reshape(8, 64, OUT).astype(np.float32)
